# revision 1
# baseline (speedup 1.0000x reference)
"""Trainium2 Bass kernel for nn_CustomGCNLayer (GCN layer, dense symmetric
adjacency from an edge list, set semantics).

Math (reference):
    h   = x @ W.T + b_lin
    A   = symmetric 0/1 adjacency from edge_index (duplicates collapse)
    out = dinv[:,None] * (A @ (dinv[:,None] * h)) + bias,
    dinv = (deg+1e-6)^-0.5

Split host/device: the host (cheap, O(N*D^2) numpy) computes
    h~ = dinv[:,None] * (x @ W.T + b_lin)        -> bf16
and the device does the O(N^2 D) aggregation:
    outT[f, i] = dinv_i * ( sum_j h~[j, f] A[j, i] + bias[f]/dinv_i )

Distribution: column shard, core k owns output rows R_k = [k*1024,(k+1)*1024);
h~ is replicated; there are NO collectives. The aggregation is 128
PSUM-accumulating matmuls (h~ 128-row blocks stationary, 0/1 adjacency tiles
moving, PE's the bottleneck at ~1 col/clk), a rank-1 bias matmul folded into
the same accumulation, then a fused DVE multiply by dinv_i and a DMA out.
The host transposes/concats the 8 outT blocks.

Adjacency tiles [128 j, 1024 i] are fed to the PE from two sources so the
combined rate keeps the PE at full clock:
  - fp8(e5m2) 0/1 dense tiles streamed from HBM (host-built; exact in fp8,
    halves the DMA bytes), split over BOTH hwdge queues (sync + scalar),
  - bf16 0/1 tiles built on the fly by the Pool engine with
    gpsimd.local_scatter from per-(j-row) destination-index lists
    (local_scatter costs num_elems*1.39ns regardless of index count, so
    Pool alone is ~2.5x too slow - that was the original bottleneck,
    along with a serial ReduceScatter tail).
"""

import dataclasses
import sys

import numpy as np

if "/opt/trn_rl_repo" not in sys.path:
    sys.path.insert(0, "/opt/trn_rl_repo")

import ml_dtypes

import concourse.bacc as bacc
import concourse.bass as bass
import concourse.mybir as mybir
import concourse.tile as tile

F32 = mybir.dt.float32
BF16 = mybir.dt.bfloat16
F8E5 = mybir.dt.float8e5
I16 = mybir.dt.int16
Alu = mybir.AluOpType
BFNP = ml_dtypes.bfloat16
F8NP = ml_dtypes.float8_e5m2


@dataclasses.dataclass(frozen=True)
class Cfg:
    N: int = 8192           # nodes
    D: int = 128            # features (in == out)
    C: int = 8              # cores
    PERIOD: int = 8         # j-block pattern period
    DMA_PER: int = 6        # blocks of each period streamed from HBM
    PADW: int = 28          # padded per-(j-row) event list width (pool blocks)

    @property
    def R(self):            # output rows per core
        return self.N // self.C

    @property
    def JB(self):           # 128-row j blocks
        return self.N // 128

    @property
    def pool_blocks(self):
        # period 0 is all-DMA so the PE can start before rc lands; period 1's
        # pool share moves to blocks 8-9, which Pool has ready early, easing
        # the DMA head rush
        return [8, 9] + [b for b in range(2 * self.PERIOD, self.JB)
                         if b % self.PERIOD >= self.DMA_PER]

    @property
    def dma_chunks(self):
        """(start_block, n_blocks) HBM-streamed chunks in consumption order,
        alternated between the two HWDGE queues by the builder. The first
        chunks are small so the PE can start as early as possible."""
        assert self.PERIOD == 8 and self.DMA_PER == 6
        chunks = [(0, 2), (2, 3), (5, 3), (10, 3), (13, 3)]
        for p in range(2, self.JB // self.PERIOD):
            chunks += [(8 * p, 3), (8 * p + 3, 3)]
        return chunks


FULL = Cfg()


def build(cfg: Cfg) -> bass.Bass:
    N, D, R, JB = cfg.N, cfg.D, cfg.R, cfg.JB
    PADW = cfg.PADW
    pool_blocks = cfg.pool_blocks
    NP = len(pool_blocks)
    tloc = {b: t for t, b in enumerate(pool_blocks)}

    nc = bacc.Bacc()

    # h~ wrapped on host: hw[p, b*128 + f] = h~[b*128 + p, f]  (bf16)
    hw = nc.dram_tensor("hw", [128, JB * D], BF16, kind="ExternalInput")
    # 0/1 adjacency columns of this core: adj[j, i] = A[i + k*R, j], fp8
    adj = nc.dram_tensor("adj", [N, R], F8E5, kind="ExternalInput")
    # per j-row destination-index lists for pool-built blocks (-1 pad)
    rc = nc.dram_tensor("rc", [128, max(1, NP) * PADW], I16,
                        kind="ExternalInput")
    outT = nc.dram_tensor("outT", [D, R], F32, kind="ExternalOutput")

    with tile.TileContext(nc, num_cores=cfg.C) as tc:
        const_p = tc.alloc_tile_pool(name="const", bufs=1)
        psum_p = tc.alloc_tile_pool(name="psum", bufs=8, space="PSUM")
        dchunk_p = tc.alloc_tile_pool(name="dchunk", bufs=8)
        ptile_p = tc.alloc_tile_pool(name="ptile", bufs=8)
        stage_p = tc.alloc_tile_pool(name="stage", bufs=1)

        # PE warm-up: the Tensor engine only reaches full clock after ~3us of
        # continuous execution. Run throwaway matmuls on memset data during
        # the DMA head so the real stream starts at full speed.
        wu_rhs = const_p.tile([128, 512], BF16, name="wu_rhs")
        nc.vector.memset(wu_rhs[:], 0.0)
        wu_ps = psum_p.tile([128, 512], F32, name="wu_ps", bufs=1)
        for w in range(14):
            nc.tensor.matmul(wu_ps[:], lhsT=wu_rhs[:, 0:128], rhs=wu_rhs[:],
                             start=(w == 0), stop=False)
        for w in range(4):
            nc.tensor.matmul(wu_ps[:, 0:128], lhsT=wu_rhs[:, 0:128],
                             rhs=wu_rhs[:, 0:128], start=False, stop=(w == 3))

        tiles = {}
        h_sb = const_p.tile([128, JB * D], BF16, name="h_sb")
        rc_sb = const_p.tile([128, NP * PADW], I16, name="rc_sb")
        HC = 16
        hchunk = JB * D // HC

        def load_h(eng, q):
            eng.dma_start(out=h_sb[:, q * hchunk:(q + 1) * hchunk],
                          in_=hw[:, q * hchunk:(q + 1) * hchunk])

        # Interleave h chunks and adjacency chunks across the two HWDGE
        # queues. First issues: the first (1-block) adjacency chunk on sync
        # and the first (4-block) h chunk on scalar, so the first matmul's
        # operands transfer concurrently and arrive in ~1us.
        hq = 1
        for ci, (s, n) in enumerate(cfg.dma_chunks):
            # blocks 0-4 all on sync; h0 + rc lead the scalar queue
            eng = nc.sync if (ci < 2 or ci % 2 == 1) else nc.scalar
            oth = nc.scalar if eng is nc.sync else nc.sync
            ch = dchunk_p.tile([128, 3 * 1024], F8E5, name="ch")
            eng.dma_start(
                out=ch[:, :n * 1024].rearrange("p (t i) -> p t i", i=1024),
                in_=adj[s * 128:(s + n) * 128, :].rearrange(
                    "(t p) i -> p t i", p=128))
            for t in range(n):
                tiles[s + t] = ch[:, t * 1024:(t + 1) * 1024]
            if ci == 0:
                load_h(nc.scalar, 0)
                nc.scalar.dma_start(out=rc_sb[:], in_=rc[:])
            elif hq < HC:
                load_h(oth, hq)
                hq += 1
        while hq < HC:
            load_h(nc.scalar if hq % 2 else nc.sync, hq)
            hq += 1

        # Pool-built 0/1 adjacency tiles (bf16, data = ones).
        ones_sb = const_p.tile([128, PADW], BF16, name="ones_sb")
        nc.vector.memset(ones_sb[:], 1.0)
        for b in pool_blocks:
            at = ptile_p.tile([128, 1024], BF16, name="pt")
            nc.gpsimd.local_scatter(
                out_ap=at[:],
                data_ap=ones_sb[:],
                idxs_ap=rc_sb[:, tloc[b] * PADW:(tloc[b] + 1) * PADW],
                channels=128,
                num_elems=R,
                num_idxs=PADW,
            )
            tiles[b] = at

        # ---- main: outT_raw[f, i] = sum_b h~blk(b).T @ adj_tile(b) --------
        ps0 = psum_p.tile([128, 512], F32, name="ps0", bufs=1)
        ps1 = psum_p.tile([128, 512], F32, name="ps1", bufs=1)
        for b in range(JB):
            hb = h_sb[:, b * D:(b + 1) * D]
            first, last = b == 0, b == JB - 1
            nc.tensor.matmul(ps0[:], lhsT=hb, rhs=tiles[b][:, 0:512],
                             start=first, stop=last)
            nc.tensor.matmul(ps1[:], lhsT=hb, rhs=tiles[b][:, 512:1024],
                             start=first, stop=last)

        # ---- tail: copy out in 256-wide pieces so the out DMAs start early;
        # the host applies dinv_i and bias --------------------------------
        o_sb = stage_p.tile([128, R], F32, name="o_sb")
        nc.vector.tensor_copy(o_sb[:, 0:256], ps0[:, 0:256])
        nc.sync.dma_start(out=outT[:, 0:256], in_=o_sb[:, 0:256])
        nc.scalar.copy(o_sb[:, 512:768], ps1[:, 0:256])
        nc.scalar.dma_start(out=outT[:, 512:768], in_=o_sb[:, 512:768])
        nc.vector.tensor_copy(o_sb[:, 256:512], ps0[:, 256:512])
        nc.sync.dma_start(out=outT[:, 256:512], in_=o_sb[:, 256:512])
        nc.scalar.copy(o_sb[:, 768:1024], ps1[:, 256:512])
        nc.scalar.dma_start(out=outT[:, 768:1024], in_=o_sb[:, 768:1024])

        for p in [stage_p, ptile_p, dchunk_p, psum_p, const_p]:
            p.release()

    return nc


def make_in_maps(cfg: Cfg, x, edge_index, W, b_lin, bias):
    N, D, C, R, JB = cfg.N, cfg.D, cfg.C, cfg.R, cfg.JB

    x = np.asarray(x, dtype=np.float32)
    W = np.asarray(W, dtype=np.float32)
    b_lin = np.asarray(b_lin, dtype=np.float32)
    bias = np.asarray(bias, dtype=np.float32)
    ei = np.asarray(edge_index).astype(np.int64)

    # symmetrize + dedup (set semantics, matches at[].set)
    key = np.unique(np.concatenate([ei[0] * N + ei[1], ei[1] * N + ei[0]]))
    de = (key // N).astype(np.int64)   # dst (output row)
    sr = (key % N).astype(np.int64)    # src
    deg = np.bincount(de, minlength=N)
    dinv = (1.0 / np.sqrt(deg.astype(np.float64) + 1e-6)).astype(np.float32)

    # h~ = dinv * (x @ W.T + b_lin), wrapped for 128-row stationary blocks
    h = (x @ W.T + b_lin) * dinv[:, None]
    hwrap = np.ascontiguousarray(
        h.astype(BFNP).reshape(JB, 128, D).transpose(1, 0, 2).reshape(
            128, JB * D))

    # pool-block event lists: group by (src row, dst core), slot = rank
    core = de // R
    jb = sr // 128
    pool_mask = np.isin(jb, np.asarray(cfg.pool_blocks))
    pe_sr, pe_de, pe_core = sr[pool_mask], de[pool_mask], core[pool_mask]
    grp = pe_sr * C + pe_core
    order = np.argsort(grp, kind="stable")
    gs = grp[order]
    cnt = np.bincount(gs, minlength=N * C)
    starts = np.concatenate([[0], np.cumsum(cnt)[:-1]])
    slot = np.arange(gs.size) - np.repeat(starts, cnt)
    padw = int(cnt.max())
    padw = max(4, (padw + 1) // 2 * 2)
    cfg = dataclasses.replace(cfg, PADW=padw)
    pool_blocks = cfg.pool_blocks
    NP = len(pool_blocks)
    tloc_arr = np.full(JB, -1, np.int64)
    for t, b in enumerate(pool_blocks):
        tloc_arr[b] = t

    o_sr, o_de, o_core = pe_sr[order], pe_de[order], pe_core[order]
    p_row = o_sr % 128
    p_t = tloc_arr[o_sr // 128]
    col = p_t * padw + slot
    rc_all = np.full((C, 128, NP * padw), -1, np.int16)
    rc_all[o_core, p_row, col] = (o_de % R).astype(np.int16)

    # dense 0/1 adjacency in fp8 e5m2 (1.0 == 0x3C), per-core column slices
    A = np.zeros((N, N), np.uint8)
    A[sr, de] = 0x3C
    A = A.view(F8NP)

    in_maps = []
    for k in range(C):
        in_maps.append({
            "hw": hwrap,
            "adj": np.ascontiguousarray(A[:, k * R:(k + 1) * R]),
            "rc": rc_all[k],
        })
    return cfg, in_maps, dinv


def kernel(x, edge_index, W, b_lin, bias, *, trace=False, cfg: Cfg = FULL):
    from concourse.bass_utils import run_bass_kernel_spmd

    if trace:
        _install_ntff_hook()
    cfg, in_maps, dinv = make_in_maps(cfg, x, edge_index, W, b_lin, bias)
    nc = build(cfg)
    nc.finalize()
    res = run_bass_kernel_spmd(nc, in_maps, core_ids=list(range(cfg.C)),
                               trace=trace)
    full = np.concatenate(
        [np.asarray(r["outT"]).T for r in res.results], axis=0)
    full = full * dinv[:, None] + np.asarray(bias, np.float32)[None, :]
    kernel.last_results = res
    return np.ascontiguousarray(full).astype(np.float32)


kernel.last_results = None


def _install_ntff_hook():
    """Provide antenv.axon_hooks (missing on this image) so that
    run_bass_kernel_spmd(trace=True) can capture NTFF profiles via the
    axon ctypes hook from trn_agent_boot."""
    import sys as _sys
    import types

    try:
        import antenv.axon_hooks  # noqa: F401
        return True
    except ImportError:
        pass
    try:
        import antenv
        from trn_agent_boot.trn_boot import _ntff_profile_via_ctypes

        hook = _ntff_profile_via_ctypes("/opt/axon/libaxon_pjrt.so")
        mod = types.ModuleType("antenv.axon_hooks")
        mod.get_axon_ntff_profile_hook = lambda: hook
        mod.set_axon_ntff_profile_hook = lambda h: None
        _sys.modules["antenv.axon_hooks"] = mod
        antenv.axon_hooks = mod
        return hook is not None
    except Exception as e:  # profiling is best-effort
        print(f"ntff hook install failed: {e}", file=sys.stderr)
        return False



# revision 6
# speedup vs baseline: 1.2557x; 1.2557x over previous
"""Trainium2 Bass kernel for nn_CustomGCNLayer (GCN layer, dense symmetric
adjacency from an edge list, set semantics).

Math (reference):
    h   = x @ W.T + b_lin
    A   = symmetric 0/1 adjacency from edge_index (duplicates collapse)
    out = dinv[:,None] * (A @ (dinv[:,None] * h)) + bias,
    dinv = (deg+1e-6)^-0.5

Host computes h~ = dinv[:,None] * (x @ W.T + b_lin) and quantizes it to
fp8e4m3 hi + lo residual. The device does the O(N^2 D) aggregation
    outT[f, i] = sum_j h~[j, f] A[j, i]
entirely with fp8 DoubleRow matmuls (2 fp8 contraction slots per PE cell
per cycle, measured ~2.3x over bf16):

  - "fast" blocks (D_FAST of 64): pairs of j-blocks share one DR matmul
    group, h in single fp8e4m3 (quantization error ~2.6% * sqrt(D_FAST/64)
    on the output, kept under the 2e-2 gate),
  - all other blocks run "hi/lo": the two DR k-slots hold fp8(h) and
    fp8(h - fp8(h)) against the SAME adjacency tile (rhs dim-1 stride 0,
    no extra bytes), giving ~bf16 accuracy at the same PE rate.

Adjacency 0/1 tiles come from two sources:
  - dense fp8e4m3 tiles streamed from HBM over both HWDGE queues
    (host-prewrapped [128, t, 1024] so descriptors are 4KB+),
  - N_POOL tiles built on-chip by gpsimd.local_scatter writing uint16
    cells (two fp8 columns per element, host pre-merges collisions),
    measured 843ns/tile vs 1229ns for bf16 tiles.

Column shard: core k owns dst rows [k*1024, (k+1)*1024); h replicated;
no collectives. Host applies dinv_i and bias and transposes/concats.
"""

import dataclasses
import sys

import numpy as np

if "/opt/trn_rl_repo" not in sys.path:
    sys.path.insert(0, "/opt/trn_rl_repo")

import ml_dtypes

import concourse.bacc as bacc
import concourse.bass as bass
import concourse.mybir as mybir
import concourse.tile as tile

F32 = mybir.dt.float32
BF16 = mybir.dt.bfloat16
F8E4 = mybir.dt.float8e4
I16 = mybir.dt.int16
DR = mybir.MatmulPerfMode.DoubleRow
BFNP = ml_dtypes.bfloat16
F8NP = ml_dtypes.float8_e4m3

ONE_E4M3 = 0x38  # fp8e4m3 bit pattern of 1.0


@dataclasses.dataclass(frozen=True)
class Cfg:
    N: int = 8192           # nodes
    D: int = 128            # features (in == out)
    C: int = 8              # cores
    D_FAST: int = 30        # j-blocks with single-fp8 h (paired in DR)
    N_POOL: int = 24        # j-blocks whose adjacency is pool-built
    PADW: int = 24          # padded per-(row, pool block) event list width
    ACH: int = 4            # adjacency tiles per DMA chunk

    @property
    def R(self):            # output rows per core
        return self.N // self.C

    @property
    def JB(self):           # 128-row j blocks
        return self.N // 128

    @property
    def N_SHILO(self):
        return self.JB - self.D_FAST - self.N_POOL

    @property
    def N_STREAM(self):
        return self.D_FAST + self.N_SHILO

    @property
    def NSLOT(self):
        return self.D_FAST // 2 + self.N_SHILO + self.N_POOL

    @property
    def h8_chunks(self):
        """k-tile counts per h8 DMA chunk (even so lhsT pairs don't straddle)."""
        K = 2 * self.NSLOT
        base = [24, 24, 24]
        base.append(K - sum(base))
        return base


FULL = Cfg()


def make_schedule(cfg: Cfg):
    """Greedy slot order: list of ('fast', t) | ('shilo', t) | ('philo', c).

    t = first streamed-tile index consumed, c = pool tile index. Streamed
    tiles are consumed in index order; fast pairs need t even (pairs must
    not straddle the ACH-tile DMA chunks).
    """
    NF, NS, NP = cfg.D_FAST // 2, cfg.N_SHILO, cfg.N_POOL
    QR = 0.18e6   # bytes/us per HWDGE queue (measured, both active)
    LAT = 0.8
    POOL_T = 0.85
    SLOT_T = 0.5

    n_ach = (cfg.N_STREAM + cfg.ACH - 1) // cfg.ACH
    adj_b = [min(cfg.ACH, cfg.N_STREAM - i * cfg.ACH) * 128 * 1024
             for i in range(n_ach)]
    h8_b = [n * 128 * 128 for n in cfg.h8_chunks]
    # use the default PADW here: the slot order must not depend on the
    # data-dependent PADW chosen later in make_in_maps
    rc_b = 2 * 128 * cfg.N_POOL * Cfg.PADW * 2

    # queue plans: (kind, idx, bytes)
    sync_q = [("rc", 0, rc_b)]
    scal_q = [("h8", 0, h8_b[0])]
    ai = hi = 0
    for ai in range(n_ach):
        (sync_q if ai % 2 == 0 else scal_q).append(("adj", ai, adj_b[ai]))
        if ai in (1, 3, 5):
            hi += 1
            if hi < len(h8_b):
                scal_q.append(("h8", hi, h8_b[hi]))
    while hi + 1 < len(h8_b):
        hi += 1
        sync_q.append(("h8", hi, h8_b[hi]))

    arr = {}
    for q in (sync_q, scal_q):
        t = LAT
        for kind, idx, b in q:
            t += b / QR
            arr[(kind, idx)] = t

    tile_arr = [arr[("adj", t // cfg.ACH)] for t in range(cfg.N_STREAM)]
    kt_chunk = []
    for j, n in enumerate(cfg.h8_chunks):
        kt_chunk += [j] * n
    pool_ready = [arr[("rc", 0)] + POOL_T * (c + 1) for c in range(NP)]

    slots = []
    t_pe = 2.0
    st = f = s = p = 0
    while f < NF or s < NS or p < NP:
        slot_idx = len(slots)
        kt_ready = arr[("h8", kt_chunk[2 * slot_idx + 1])]
        cands = []
        if f < NF and st % 2 == 0:
            cands.append(("fast", max(tile_arr[st + 1], kt_ready), NF - f, 1.0))
        if s < NS:
            cands.append(("shilo", max(tile_arr[st], kt_ready), NS - s, 0.5))
        if p < NP:
            cands.append(("philo", max(pool_ready[p], kt_ready),
                          NP - p, POOL_T))
        # earliest-ready first; tie-break toward the most backlogged source
        cands.sort(key=lambda x: (max(t_pe, x[1]), -x[2] * x[3]))
        typ, rdy, _, _ = cands[0]
        if typ == "fast":
            slots.append(("fast", st)); st += 2; f += 1
        elif typ == "shilo":
            slots.append(("shilo", st)); st += 1; s += 1
        else:
            slots.append(("philo", p)); p += 1
        t_pe = max(t_pe, rdy) + SLOT_T
    return slots, sync_q, scal_q


def build(cfg: Cfg) -> bass.Bass:
    R, PADW = cfg.R, cfg.PADW
    slots, sync_q, scal_q = make_schedule(cfg)
    K = 2 * cfg.NSLOT
    n_ach = (cfg.N_STREAM + cfg.ACH - 1) // cfg.ACH

    nc = bacc.Bacc()
    adjw = nc.dram_tensor("adjw", [128, cfg.N_STREAM * 1024], F8E4,
                          kind="ExternalInput")
    hs8 = nc.dram_tensor("hs8", [128, K * cfg.D], F8E4, kind="ExternalInput")
    rcv = nc.dram_tensor("rcv", [128, max(1, 2 * cfg.N_POOL * PADW)], I16,
                         kind="ExternalInput")
    outT = nc.dram_tensor("outT", [cfg.D, R], BF16, kind="ExternalOutput")

    with tile.TileContext(nc, num_cores=cfg.C) as tc:
        const_p = tc.alloc_tile_pool(name="const", bufs=1)
        psum_p = tc.alloc_tile_pool(name="psum", bufs=8, space="PSUM")

        # PE warm-up on memset data: the Tensor engine reaches full clock
        # only after ~3us of continuous execution.
        wu = const_p.tile([128, 512], BF16, name="wu")
        nc.vector.memset(wu[:], 0.0)
        wp = psum_p.tile([128, 512], F32, name="wp", bufs=1)
        for w in range(14):
            nc.tensor.matmul(wp[:], lhsT=wu[:, 0:128], rhs=wu[:],
                             start=(w == 0), stop=False)
        for w in range(4):
            nc.tensor.matmul(wp[:, 0:128], lhsT=wu[:, 0:128],
                             rhs=wu[:, 0:128], start=False, stop=(w == 3))

        # SBUF tiles (all resident)
        rc_sb = const_p.tile([128, max(1, 2 * cfg.N_POOL * PADW)], I16,
                             name="rc_sb")
        h8_t = []
        off = 0
        for j, nk in enumerate(cfg.h8_chunks):
            h8_t.append((const_p.tile([128, nk, cfg.D], F8E4, name=f"h8_{j}"),
                         off, nk))
            off += nk
        adj_t = []
        for ai in range(n_ach):
            n = min(cfg.ACH, cfg.N_STREAM - ai * cfg.ACH)
            adj_t.append((const_p.tile([128, n, 1024], F8E4, name=f"adj_{ai}"),
                          ai * cfg.ACH, n))
        pool_t = [const_p.tile([128, 1024], F8E4, name=f"pool_{c}")
                  for c in range(cfg.N_POOL)]
        o_sb = const_p.tile([128, R], BF16, name="o_sb")

        # DMA issue per queue plan
        def issue(eng, kind, idx):
            if kind == "rc":
                eng.dma_start(out=rc_sb[:], in_=rcv[:])
            elif kind == "h8":
                t, off, nk = h8_t[idx]
                eng.dma_start(
                    out=t[:],
                    in_=hs8[:, off * cfg.D:(off + nk) * cfg.D].rearrange(
                        "p (t m) -> p t m", m=cfg.D))
            else:
                t, off, n = adj_t[idx]
                eng.dma_start(
                    out=t[:],
                    in_=adjw[:, off * 1024:(off + n) * 1024].rearrange(
                        "p (t i) -> p t i", i=1024))

        for kind, idx, _ in sync_q:
            issue(nc.sync, kind, idx)
        for kind, idx, _ in scal_q:
            issue(nc.scalar, kind, idx)

        # pool-built adjacency tiles: uint16 cells = 2 fp8 columns
        for c in range(cfg.N_POOL):
            nc.gpsimd.local_scatter(
                out_ap=pool_t[c].bitcast(I16)[:],
                data_ap=rc_sb[:, (2 * c + 1) * PADW:(2 * c + 2) * PADW],
                idxs_ap=rc_sb[:, (2 * c) * PADW:(2 * c + 1) * PADW],
                channels=128,
                num_elems=512,
                num_idxs=PADW,
            )

        # main DR matmul stream
        ps0 = psum_p.tile([128, 512], F32, name="ps0", bufs=1)
        ps1 = psum_p.tile([128, 512], F32, name="ps1", bufs=1)

        def kt_ap(slot_idx):
            kt = 2 * slot_idx
            for t, off, nk in h8_t:
                if off <= kt < off + nk:
                    return t[:, kt - off:kt - off + 2, :]
            raise AssertionError

        def adj_ap(t0, n):
            for t, off, nt in adj_t:
                if off <= t0 < off + nt:
                    assert t0 + n <= off + nt
                    return t[:, t0 - off:t0 - off + n, :]
            raise AssertionError

        for i, (typ, arg) in enumerate(slots):
            first, last = i == 0, i == len(slots) - 1
            lhsT = kt_ap(i)
            if typ == "fast":
                rhs_full = adj_ap(arg, 2)
            elif typ == "shilo":
                rhs_full = adj_ap(arg, 1).to_broadcast((128, 2, 1024))
            else:
                rhs_full = pool_t[arg][:, None, :].to_broadcast((128, 2, 1024))
            for m in range(4):
                pst = (ps0 if m < 2 else ps1)[:, (m % 2) * 256:(m % 2) * 256 + 256]
                nc.tensor.matmul(
                    pst, lhsT=lhsT,
                    rhs=rhs_full[:, :, m * 256:(m + 1) * 256],
                    # start resets the whole PSUM bank, so only the first
                    # matmul into each bank may carry it
                    start=first and m % 2 == 0, stop=last, perf_mode=DR)

        # tail: f32 PSUM -> bf16 SBUF -> HBM, split across engines/queues
        nc.vector.tensor_copy(o_sb[:, 0:512], ps0[:])
        nc.sync.dma_start(out=outT[:, 0:512], in_=o_sb[:, 0:512])
        nc.scalar.copy(o_sb[:, 512:1024], ps1[:])
        nc.scalar.dma_start(out=outT[:, 512:1024], in_=o_sb[:, 512:1024])

        psum_p.release()
        const_p.release()

    return nc


def make_in_maps(cfg: Cfg, x, edge_index, W, b_lin, bias):
    N, D, C, R = cfg.N, cfg.D, cfg.C, cfg.R

    x = np.asarray(x, dtype=np.float32)
    W = np.asarray(W, dtype=np.float32)
    b_lin = np.asarray(b_lin, dtype=np.float32)
    ei = np.asarray(edge_index).astype(np.int64)

    # symmetrize + dedup (set semantics, matches at[].set)
    key = np.unique(np.concatenate([ei[0] * N + ei[1], ei[1] * N + ei[0]]))
    sr = (key // N).astype(np.int64)   # src row of A (first index)
    de = (key % N).astype(np.int64)    # dst col
    deg = np.bincount(sr, minlength=N)
    dinv = (1.0 / np.sqrt(deg.astype(np.float64) + 1e-6)).astype(np.float32)

    # h~ = dinv * (x @ W.T + b_lin); hi/lo fp8 split
    h = (x @ W.T + b_lin) * dinv[:, None]
    hi = h.astype(F8NP)
    lo = (h - hi.astype(np.float32)).astype(F8NP)

    # block roles: streamed tiles consume j-blocks 0..N_STREAM-1 in order,
    # pool tile c covers j-block N_STREAM + c
    slots, _, _ = make_schedule(cfg)

    # h8 k-tile stream in slot order
    kts = []
    for typ, arg in slots:
        if typ == "fast":
            kts += [hi[(arg) * 128:(arg + 1) * 128],
                    hi[(arg + 1) * 128:(arg + 2) * 128]]
        elif typ == "shilo":
            kts += [hi[arg * 128:(arg + 1) * 128],
                    lo[arg * 128:(arg + 1) * 128]]
        else:
            b = cfg.N_STREAM + arg
            kts += [hi[b * 128:(b + 1) * 128],
                    lo[b * 128:(b + 1) * 128]]
    hs8 = np.ascontiguousarray(
        np.stack(kts).transpose(1, 0, 2)).reshape(128, -1)

    # dense adjacency byte matrix (0x38 = fp8e4m3 1.0)
    A = np.zeros((N, N), np.uint8)
    A[sr, de] = ONE_E4M3

    # pool events: j-blocks >= N_STREAM, merged into uint16 cells
    pool_lo = cfg.N_STREAM * 128
    pm = sr >= pool_lo
    p_sr, p_de = sr[pm], de[pm]
    core = p_de // R
    c = (p_sr - pool_lo) // 128
    row = p_sr % 128
    cell = (p_de % R) >> 1
    half = (p_de % R) & 1
    gkey = (((core * cfg.N_POOL + c) * 128 + row) * 512 + cell).astype(np.int64)
    order = np.argsort(gkey, kind="stable")
    gs = gkey[order]
    vals = (ONE_E4M3 << (8 * half[order])).astype(np.uint16)
    uk, starts = np.unique(gs, return_index=True)
    merged = np.bitwise_or.reduceat(vals, starts)
    grp = uk // 512
    cnt = np.bincount(grp, minlength=max(1, C * cfg.N_POOL * 128))
    padw = int(cnt.max()) if cnt.size else 4
    padw = max(4, (padw + 1) // 2 * 2)
    cfg = dataclasses.replace(cfg, PADW=padw)
    g_start = np.concatenate([[0], np.cumsum(cnt)[:-1]])
    slot_in_g = np.arange(uk.size) - g_start[grp]
    g_core = grp // (cfg.N_POOL * 128)
    g_c = (grp // 128) % cfg.N_POOL
    g_row = grp % 128
    # rcv layout per core: [128, (idx block c | val block c) * N_POOL * PADW]
    rcv_all = np.full((C, 128, max(1, 2 * cfg.N_POOL * padw)), -1, np.int16)
    if uk.size:
        rcv_all[g_core, g_row, (2 * g_c) * padw + slot_in_g] = (
            uk % 512).astype(np.int16)
        rcv_all[g_core, g_row, (2 * g_c + 1) * padw + slot_in_g] = (
            merged.astype(np.int16))

    in_maps = []
    for k in range(C):
        sl = A[:cfg.N_STREAM * 128, k * R:(k + 1) * R]
        adjw = np.ascontiguousarray(
            sl.reshape(cfg.N_STREAM, 128, R).transpose(1, 0, 2)
        ).reshape(128, -1).view(F8NP)
        in_maps.append({
            "adjw": adjw,
            "hs8": hs8.view(F8NP),
            "rcv": rcv_all[k],
        })
    return cfg, in_maps, dinv


def kernel(x, edge_index, W, b_lin, bias, *, trace=False, cfg: Cfg = FULL):
    from concourse.bass_utils import run_bass_kernel_spmd

    if trace:
        _install_ntff_hook()
    cfg, in_maps, dinv = make_in_maps(cfg, x, edge_index, W, b_lin, bias)
    nc = build(cfg)
    nc.finalize()
    res = run_bass_kernel_spmd(nc, in_maps, core_ids=list(range(cfg.C)),
                               trace=trace)
    full = np.concatenate(
        [np.asarray(r["outT"]).astype(np.float32).T for r in res.results],
        axis=0)
    full = full * dinv[:, None] + np.asarray(bias, np.float32)[None, :]
    kernel.last_results = res
    return np.ascontiguousarray(full).astype(np.float32)


kernel.last_results = None


def _install_ntff_hook():
    """Provide antenv.axon_hooks (missing on this image) so that
    run_bass_kernel_spmd(trace=True) can capture NTFF profiles via the
    axon ctypes hook from trn_agent_boot."""
    import sys as _sys
    import types

    try:
        import antenv.axon_hooks  # noqa: F401
        return True
    except ImportError:
        pass
    try:
        import antenv
        from trn_agent_boot.trn_boot import _ntff_profile_via_ctypes

        hook = _ntff_profile_via_ctypes("/opt/axon/libaxon_pjrt.so")
        mod = types.ModuleType("antenv.axon_hooks")
        mod.get_axon_ntff_profile_hook = lambda: hook
        mod.set_axon_ntff_profile_hook = lambda h: None
        _sys.modules["antenv.axon_hooks"] = mod
        antenv.axon_hooks = mod
        return hook is not None
    except Exception as e:  # profiling is best-effort
        print(f"ntff hook install failed: {e}", file=sys.stderr)
        return False
